# revision 12
# baseline (speedup 1.0000x reference)
"""Trainium2 Bass kernel for nn_CausalLinearSelfAttention_30013231464545.

Math note: the reference cumsums the [B,T,H,D,M] kv tensor over axis=-2,
which is the *D* axis (faithful to the original torch code), so
  kv_sum[b,t,h,d,m] = csD(kf)[b,t,h,d] * v[b,t,h,m]
and the whole module collapses to
  out[b,t,h,m] = (s / denom) * v[b,t,h,m]
with
  denom[b,t,h] = sum_d qf * cumsum_T(kf)      (true causal running key sum)
  s[b,t,h]     = sum_d qf * cumsum_D(kf)      (per-timestep D-prefix sum)
  qf = elu(q)+1 = relu(q) + exp(min(q,0)),  kf likewise.
(Validated vs the jax reference at rel err 2e-7.)

Sharding: B*H = 16 (b,h) slices; each core takes one (b, head-pair) slice
[T=2048, 2*64] so DMA rows are 512B contiguous. No cross-core comm.

Per-core dataflow (T on partitions, (h,d) on free):
  - feature maps: ACT exp + DVE scalar_tensor_tensor (relu(x)+e^x)
  - cumsum over T: PE triangular matmul per 128-tile + all-ones matmuls for
    in-group tile carries + rank-1 matmul broadcasting the cross-group carry
  - cumsum over D: DVE tensor_tensor_scan per 64-wide segment
  - dots over D: DVE tensor_tensor_reduce (fused mul+row-reduce)
  - scale & output: DVE divide + broadcast multiply
"""

import numpy as np
import sys

sys.path.insert(0, "/opt/trn_rl_repo")

B, T, H, D = 2, 2048, 8, 64
P = 128          # partitions (t per tile)
HPC = 2          # heads per core
C = HPC * D      # per-core free width = 128
NT = T // P      # 16 t-tiles per core
GT = 4           # t-tiles per group
NG = NT // GT    # 4 groups
FD = GT * C      # 512 free elements per group supertile
NSEG = GT * HPC  # 8 (tile, head) segments per group

_CACHE = {}


def _build_nc():
    import concourse.bass as bass
    import concourse.bacc as bacc
    import concourse.mybir as mybir
    from concourse import tile

    dt = mybir.dt
    f32 = dt.float32
    Alu = mybir.AluOpType
    Act = mybir.ActivationFunctionType

    nc = bacc.Bacc(None)

    q_d = nc.declare_dram_parameter("q", [T, C], f32, isOutput=False)
    k_d = nc.declare_dram_parameter("k", [T, C], f32, isOutput=False)
    v_d = nc.declare_dram_parameter("v", [T, C], f32, isOutput=False)
    o_d = nc.declare_dram_parameter("o", [T, C], f32, isOutput=True)

    # tri[t', t] = 1 if t' <= t  (lhsT for in-tile cumsum over partitions)
    tri_np = np.triu(np.ones((P, P), dtype=np.float32))
    tri_d = nc.inline_tensor(tri_np, name="tri_const")

    with tile.TileContext(nc) as tc:
        with (
            tc.tile_pool(name="const", bufs=1) as cpool,
            tc.tile_pool(name="io", bufs=3) as io,
            tc.tile_pool(name="wk", bufs=2) as wk,
            tc.tile_pool(name="cr", bufs=2) as crp,
            tc.tile_pool(name="ps", bufs=2, space="PSUM") as pp,
        ):
            tri_t = cpool.tile([P, P], f32, tag="tri")
            nc.sync.dma_start(tri_t[:], tri_d[:])
            ones_t = cpool.tile([P, P], f32, tag="ones")
            nc.vector.memset(ones_t[:], 1.0)
            onerow_t = cpool.tile([1, P], f32, tag="onerow")
            nc.vector.memset(onerow_t[:], 1.0)

            # cs accumulates column sums of all kf tiles seen so far (PSUM,
            # single accumulation group held open across groups); carry_g is
            # its SBUF snapshot before group g's tiles are added.
            cs = pp.tile([1, C], f32, tag="cs")
            cs_started = False
            carry = None  # [1, C] SBUF snapshot: sum of kf over previous groups

            for g in range(NG):
                rows = slice(g * GT * P, (g + 1) * GT * P)
                qv = q_d[rows, :].rearrange("(j p) c -> p j c", p=P)
                kv = k_d[rows, :].rearrange("(j p) c -> p j c", p=P)
                vv = v_d[rows, :].rearrange("(j p) c -> p j c", p=P)

                qt = io.tile([P, FD], f32, tag="q")
                kt = io.tile([P, FD], f32, tag="k")
                vt = io.tile([P, FD], f32, tag="v")
                nc.sync.dma_start(qt[:].rearrange("p (j c) -> p j c", c=C), qv)
                nc.sync.dma_start(kt[:].rearrange("p (j c) -> p j c", c=C), kv)
                nc.sync.dma_start(vt[:].rearrange("p (j c) -> p j c", c=C), vv)

                # feature maps: f = relu(x) + exp(min(x, 0))
                mq = wk.tile([P, FD], f32, tag="mq")
                mk = wk.tile([P, FD], f32, tag="mk")
                nc.gpsimd.tensor_scalar_min(mq[:], qt[:], 0.0)
                nc.gpsimd.tensor_scalar_min(mk[:], kt[:], 0.0)
                eq = wk.tile([P, FD], f32, tag="eq")
                ek = wk.tile([P, FD], f32, tag="ek")
                nc.scalar.activation(eq[:], mq[:], Act.Exp)
                nc.scalar.activation(ek[:], mk[:], Act.Exp)
                qf = wk.tile([P, FD], f32, tag="qf")
                kf = wk.tile([P, FD], f32, tag="kf")
                nc.vector.scalar_tensor_tensor(
                    qf[:], qt[:], 0.0, eq[:], op0=Alu.max, op1=Alu.add
                )
                nc.vector.scalar_tensor_tensor(
                    kf[:], kt[:], 0.0, ek[:], op0=Alu.max, op1=Alu.add
                )

                # snapshot the cross-group carry (column sums of all previous
                # groups' kf) into SBUF before this group's sums are added
                if g > 0:
                    carry = crp.tile([1, C], f32, tag="carry")
                    nc.scalar.copy(carry[:], cs[:])
                # cs := carry + this group's per-tile column sums (fresh
                # accumulation group per iteration; sim forbids reading PSUM
                # mid-group, so re-add the snapshot instead of holding it open)
                if g < NG - 1:
                    if g > 0:
                        nc.tensor.matmul(
                            cs[:], onerow_t[:, 0:1], carry[:],
                            start=True, stop=False,
                        )
                    for j in range(GT):
                        sl = slice(j * C, (j + 1) * C)
                        nc.tensor.matmul(
                            cs[:], ones_t[:, 0:1], kf[:, sl],
                            start=(g == 0 and j == 0), stop=(j == GT - 1),
                        )

                # ---- cumsum over T into PSUM: ks[p, j*C+c] = sum_{t'<=t} kf ----
                # One accumulation group for the whole bank (PSUM zero regions
                # are 2KB = the full bank row): first matmul start=True, last
                # stop=True; PE executes same-tile matmuls in emission order.
                ks = pp.tile([P, FD], f32, tag="ks")
                mms = []
                # in-tile cumsum
                for j in range(GT):
                    sl = slice(j * C, (j + 1) * C)
                    mms.append((sl, tri_t[:], kf[:, sl]))
                # in-group carries: column sums of earlier tiles, broadcast rows
                for jp in range(GT - 1):
                    slp = slice(jp * C, (jp + 1) * C)
                    for j in range(jp + 1, GT):
                        sl = slice(j * C, (j + 1) * C)
                        mms.append((sl, ones_t[:], kf[:, slp]))
                # cross-group carry: rank-1 broadcast of carry row
                if g > 0:
                    for j in range(GT):
                        sl = slice(j * C, (j + 1) * C)
                        mms.append((sl, onerow_t[:], carry[:]))
                for i, (sl, lhsT, rhs) in enumerate(mms):
                    nc.tensor.matmul(
                        ks[:, sl], lhsT, rhs,
                        start=(i == 0), stop=(i == len(mms) - 1),
                    )
                # ---- cumsum over D (per 64-wide segment) ----
                csD = wk.tile([P, FD], f32, tag="csD")
                for s in range(NSEG):
                    sl = slice(s * D, (s + 1) * D)
                    nc.vector.tensor_tensor_scan(
                        csD[:, sl], kf[:, sl], kf[:, sl], 0.0,
                        op0=Alu.add, op1=Alu.bypass,
                    )

                # ---- dots over D: denom & s per (tile, head) segment ----
                # (tensor_tensor_reduce compiles but crashes at runtime on HW;
                # use TT-mult + multi-axis tensor_reduce instead)
                dn = wk.tile([P, 2 * NSEG], f32, tag="dn")
                scr = wk.tile([P, FD], f32, tag="scr")
                scr2 = wk.tile([P, FD], f32, tag="scr2")
                nc.vector.tensor_tensor(scr[:], qf[:], ks[:], op=Alu.mult)
                nc.vector.tensor_reduce(
                    dn[:, 0:NSEG],
                    scr[:].rearrange("p (s d) -> p s d", d=D),
                    axis=mybir.AxisListType.X, op=Alu.add,
                )
                nc.vector.tensor_tensor(scr2[:], qf[:], csD[:], op=Alu.mult)
                nc.vector.tensor_reduce(
                    dn[:, NSEG : 2 * NSEG],
                    scr2[:].rearrange("p (s d) -> p s d", d=D),
                    axis=mybir.AxisListType.X, op=Alu.add,
                )

                # scale[t, seg] = s / denom
                rec = wk.tile([P, NSEG], f32, tag="rec")
                nc.vector.reciprocal(rec[:], dn[:, 0:NSEG])
                sc = wk.tile([P, NSEG], f32, tag="sc")
                nc.vector.tensor_tensor(
                    sc[:], dn[:, NSEG : 2 * NSEG], rec[:], op=Alu.mult
                )

                # out = v * scale (broadcast over the 64-wide segment)
                ot = io.tile([P, FD], f32, tag="o")
                sc_b = sc[:].rearrange("p (s one) -> p s one", one=1).broadcast_to(
                    [P, NSEG, D]
                )
                nc.vector.tensor_tensor(
                    ot[:].rearrange("p (s d) -> p s d", d=D),
                    vt[:].rearrange("p (s d) -> p s d", d=D),
                    sc_b,
                    op=Alu.mult,
                )
                ov = o_d[rows, :].rearrange("(j p) c -> p j c", p=P)
                nc.sync.dma_start(ov, ot[:].rearrange("p (j c) -> p j c", c=C))

    nc.compile()
    return nc


def get_nc():
    if "nc" not in _CACHE:
        _CACHE["nc"] = _build_nc()
    return _CACHE["nc"]


def shard_inputs(q, k, v):
    """core c -> (b = c//4, heads 2*(c%4), 2*(c%4)+1); returns list of in_maps."""
    maps = []
    for c in range(8):
        b, hp = divmod(c, 4)
        hs = slice(2 * hp, 2 * hp + 2)
        maps.append(
            {
                "q": np.ascontiguousarray(q[b, :, hs, :].reshape(T, C)),
                "k": np.ascontiguousarray(k[b, :, hs, :].reshape(T, C)),
                "v": np.ascontiguousarray(v[b, :, hs, :].reshape(T, C)),
            }
        )
    return maps


def gather_outputs(results):
    out = np.empty((B, T, H, D), dtype=np.float32)
    for c in range(8):
        b, hp = divmod(c, 4)
        out[b, :, 2 * hp : 2 * hp + 2, :] = results[c]["o"].reshape(T, HPC, D)
    return out


def kernel(q, k, v):
    from concourse.bass_utils import run_bass_kernel_spmd

    q = np.asarray(q, dtype=np.float32)
    k = np.asarray(k, dtype=np.float32)
    v = np.asarray(v, dtype=np.float32)
    nc = get_nc()
    maps = shard_inputs(q, k, v)
    res = run_bass_kernel_spmd(nc, maps, list(range(8)))
    return gather_outputs(res.results)


# revision 14
# speedup vs baseline: 2.0046x; 2.0046x over previous
"""Trainium2 Bass kernel for nn_CausalLinearSelfAttention_30013231464545.

Math note: the reference cumsums the [B,T,H,D,M] kv tensor over axis=-2,
which is the *D* axis (faithful to the original torch code), so
  kv_sum[b,t,h,d,m] = csD(kf)[b,t,h,d] * v[b,t,h,m]
and the whole module collapses to
  out[b,t,h,m] = (s / denom) * v[b,t,h,m]
with
  denom[b,t,h] = sum_d qf * cumsum_T(kf)      (true causal running key sum)
  s[b,t,h]     = sum_d qf * cumsum_D(kf)      (per-timestep D-prefix sum)
  qf = elu(q)+1 = min(exp(q), 1) + relu(q),  kf likewise.
(Validated vs the jax reference at rel err 2e-7 in fp32.)

Sharding: B*H = 16 (b,h) slices; each core takes one (b, head-pair) slice
[T=2048, 2*64] so DMA rows are 512B contiguous. No cross-core comm.

Per-core dataflow (T on partitions, (h,d) on free; fp16 on-chip, fp32 I/O):
  - feature maps: ACT exp -> DVE min/relu/add (all 16-bit fast modes)
  - cumsum over T: PE triangular matmul per 128-tile + all-ones matmuls for
    in-group tile carries + rank-1 matmul broadcasting the cross-group carry
    (carry maintained by tiny column-sum matmuls into a [1,C] PSUM tile)
  - cumsum over D: PE transpose of kf tiles + block-upper-triangular matmul
  - dots over D: DVE fp16 TT multiplies + one multi-axis tensor_reduce
  - scale & output: DVE reciprocal + per-segment tensor_scalar multiplies
"""

import numpy as np
import sys

sys.path.insert(0, "/opt/trn_rl_repo")

B, T, H, D = 2, 2048, 8, 64
P = 128          # partitions (t per tile)
HPC = 2          # heads per core
C = HPC * D      # per-core free width = 128
NT = T // P      # 16 t-tiles per core
GT = 4           # t-tiles per group
NG = NT // GT    # 4 groups
FD = GT * C      # 512 free elements per group supertile
NSEG = GT * HPC  # 8 (tile, head) segments per group

_CACHE = {}


def _build_nc():
    import concourse.bass as bass
    import concourse.bacc as bacc
    import concourse.mybir as mybir
    from concourse import tile

    dt = mybir.dt
    f32 = dt.float32
    f16 = dt.float16
    Alu = mybir.AluOpType
    Act = mybir.ActivationFunctionType

    nc = bacc.Bacc(None)

    q_d = nc.declare_dram_parameter("q", [T, C], f32, isOutput=False)
    k_d = nc.declare_dram_parameter("k", [T, C], f32, isOutput=False)
    v_d = nc.declare_dram_parameter("v", [T, C], f32, isOutput=False)
    o_d = nc.declare_dram_parameter("o", [T, C], f32, isOutput=True)

    # tri[t', t] = 1 if t' <= t  (lhsT for in-tile cumsum over partitions)
    tri_d = nc.inline_tensor(
        np.triu(np.ones((P, P), dtype=np.float16)), name="tri_const"
    )
    # identity for PE transpose
    eye_d = nc.inline_tensor(np.eye(P, dtype=np.float16), name="eye_const")
    # block-diag of two 64x64 upper-tri-ones: csD[t, (h,d2)] = sum_{d<=d2} kf
    mblk_np = np.zeros((P, P), dtype=np.float16)
    u64 = np.triu(np.ones((D, D), dtype=np.float16))
    mblk_np[:D, :D] = u64
    mblk_np[D:, D:] = u64
    mblk_d = nc.inline_tensor(mblk_np, name="mblk_const")

    with tile.TileContext(nc) as tc:
        with (
            tc.tile_pool(name="const", bufs=1) as cpool,
            tc.tile_pool(name="io", bufs=3) as io,
            tc.tile_pool(name="wk", bufs=2) as wk,
            tc.tile_pool(name="cr", bufs=2) as crp,
            tc.tile_pool(name="ps", bufs=2, space="PSUM") as pp,
            tc.tile_pool(name="ps1", bufs=1, space="PSUM") as pp1,
        ):
            tri_t = cpool.tile([P, P], f16, tag="tri")
            nc.sync.dma_start(tri_t[:], tri_d[:])
            eye_t = cpool.tile([P, P], f16, tag="eye")
            nc.sync.dma_start(eye_t[:], eye_d[:])
            mblk_t = cpool.tile([P, P], f16, tag="mblk")
            nc.sync.dma_start(mblk_t[:], mblk_d[:])
            ones_t = cpool.tile([P, P], f16, tag="ones")
            nc.vector.memset(ones_t[:], 1.0)
            onerow_t = cpool.tile([1, P], f16, tag="onerow")
            nc.vector.memset(onerow_t[:], 1.0)

            # cs: [1, C] PSUM running column-sums of kf (re-seeded per group);
            # carry_g = SBUF fp16 snapshot before group g's tiles are added.
            cs = pp1.tile([1, C], f32, tag="cs")
            carry = None

            for g in range(NG):
                rows = slice(g * GT * P, (g + 1) * GT * P)
                qv = q_d[rows, :].rearrange("(j p) c -> p j c", p=P)
                kv = k_d[rows, :].rearrange("(j p) c -> p j c", p=P)
                vv = v_d[rows, :].rearrange("(j p) c -> p j c", p=P)

                qt = io.tile([P, FD], f32, tag="q")
                kt = io.tile([P, FD], f32, tag="k")
                vt = io.tile([P, FD], f32, tag="v")
                nc.sync.dma_start(qt[:].rearrange("p (j c) -> p j c", c=C), qv)
                nc.sync.dma_start(kt[:].rearrange("p (j c) -> p j c", c=C), kv)
                nc.sync.dma_start(vt[:].rearrange("p (j c) -> p j c", c=C), vv)

                # feature maps: f = min(exp(x), 1) + relu(x)  (== elu(x)+1)
                eq = wk.tile([P, FD], f16, tag="eq")
                ek = wk.tile([P, FD], f16, tag="ek")
                nc.scalar.activation(eq[:], qt[:], Act.Exp)
                nc.scalar.activation(ek[:], kt[:], Act.Exp)
                emq = wk.tile([P, FD], f16, tag="emq")
                emk = wk.tile([P, FD], f16, tag="emk")
                nc.vector.tensor_scalar_min(emq[:], eq[:], 1.0)
                nc.vector.tensor_scalar_min(emk[:], ek[:], 1.0)
                rq = wk.tile([P, FD], f16, tag="rq")
                rk = wk.tile([P, FD], f16, tag="rk")
                nc.vector.tensor_scalar_max(rq[:], qt[:], 0.0)
                nc.vector.tensor_scalar_max(rk[:], kt[:], 0.0)
                qf = wk.tile([P, FD], f16, tag="qf")
                kf = wk.tile([P, FD], f16, tag="kf")
                nc.vector.tensor_tensor(qf[:], emq[:], rq[:], op=Alu.add)
                nc.vector.tensor_tensor(kf[:], emk[:], rk[:], op=Alu.add)

                # snapshot cross-group carry, then cs := carry + group colsums
                if g > 0:
                    carry = crp.tile([1, C], f16, tag="carry")
                    nc.scalar.copy(carry[:], cs[:])
                if g < NG - 1:
                    if g > 0:
                        nc.tensor.matmul(
                            cs[:], onerow_t[:, 0:1], carry[:],
                            start=True, stop=False,
                        )
                    for j in range(GT):
                        sl = slice(j * C, (j + 1) * C)
                        nc.tensor.matmul(
                            cs[:], ones_t[:, 0:1], kf[:, sl],
                            start=(g == 0 and j == 0), stop=(j == GT - 1),
                        )

                # ---- cumsum over T into PSUM (one accumulation group/bank) --
                ks = pp.tile([P, FD], f32, tag="ks")
                mms = []
                for j in range(GT):
                    sl = slice(j * C, (j + 1) * C)
                    mms.append((sl, tri_t[:], kf[:, sl]))
                # in-group carries: colsums of tile jp broadcast to tiles >jp
                # (single matmul per jp: rhs 0-stride-broadcast across slices)
                for jp in range(GT - 1):
                    n = GT - 1 - jp
                    sl = slice((jp + 1) * C, GT * C)
                    rhs = (
                        kf[:, jp * C : (jp + 1) * C]
                        .rearrange("p (one c) -> p one c", one=1)
                        .broadcast_to([P, n, C])
                    )
                    mms.append((sl, ones_t[:], rhs))
                if g > 0:
                    rhs = (
                        carry[:]
                        .rearrange("p (one c) -> p one c", one=1)
                        .broadcast_to([1, GT, C])
                    )
                    mms.append((slice(0, FD), onerow_t[:], rhs))
                for i, (sl, lhsT, rhs) in enumerate(mms):
                    nc.tensor.matmul(
                        ks[:, sl], lhsT, rhs,
                        start=(i == 0), stop=(i == len(mms) - 1),
                    )
                ks_sb = wk.tile([P, FD], f16, tag="ks_sb")
                nc.scalar.copy(ks_sb[:], ks[:])

                # ---- cumsum over D via PE: transpose kf, then Mblk matmul --
                kT = pp.tile([P, FD], f16, tag="kT")
                for j in range(GT):
                    sl = slice(j * C, (j + 1) * C)
                    nc.tensor.matmul(
                        kT[:, sl], kf[:, sl], eye_t[:], is_transpose=True,
                        start=(j == 0), stop=(j == GT - 1),
                    )
                kfT = wk.tile([P, FD], f16, tag="kfT")
                nc.scalar.copy(kfT[:], kT[:])
                csD = pp.tile([P, FD], f32, tag="csD")
                for j in range(GT):
                    sl = slice(j * C, (j + 1) * C)
                    nc.tensor.matmul(
                        csD[:, sl], kfT[:, sl], mblk_t[:],
                        start=(j == 0), stop=(j == GT - 1),
                    )
                csD_sb = wk.tile([P, FD], f16, tag="csD_sb")
                nc.scalar.copy(csD_sb[:], csD[:])

                # ---- dots over D: one big fp16 mult + one multi-axis reduce -
                scr = wk.tile([P, 2 * FD], f16, tag="scr")
                nc.vector.tensor_tensor(scr[:, 0:FD], qf[:], ks_sb[:], op=Alu.mult)
                nc.vector.tensor_tensor(
                    scr[:, FD : 2 * FD], qf[:], csD_sb[:], op=Alu.mult
                )
                dn = wk.tile([P, 2 * NSEG], f32, tag="dn")
                nc.vector.tensor_reduce(
                    dn[:],
                    scr[:].rearrange("p (s d) -> p s d", d=D),
                    axis=mybir.AxisListType.X, op=Alu.add,
                )

                # scale[t, seg] = s / denom
                rec = wk.tile([P, NSEG], f32, tag="rec")
                nc.vector.reciprocal(rec[:], dn[:, 0:NSEG])
                sc = wk.tile([P, NSEG], f32, tag="sc")
                nc.vector.tensor_tensor(
                    sc[:], dn[:, NSEG : 2 * NSEG], rec[:], op=Alu.mult
                )

                # out = v * scale (per-segment scalar multiply, fp32 2x mode)
                ot = io.tile([P, FD], f32, tag="o")
                for s in range(NSEG):
                    sl = slice(s * D, (s + 1) * D)
                    nc.vector.tensor_scalar_mul(
                        ot[:, sl], vt[:, sl], sc[:, s : s + 1]
                    )
                ov = o_d[rows, :].rearrange("(j p) c -> p j c", p=P)
                nc.sync.dma_start(ov, ot[:].rearrange("p (j c) -> p j c", c=C))

    nc.compile()
    return nc


def get_nc():
    if "nc" not in _CACHE:
        _CACHE["nc"] = _build_nc()
    return _CACHE["nc"]


def shard_inputs(q, k, v):
    """core c -> (b = c//4, heads 2*(c%4), 2*(c%4)+1); returns list of in_maps."""
    maps = []
    for c in range(8):
        b, hp = divmod(c, 4)
        hs = slice(2 * hp, 2 * hp + 2)
        maps.append(
            {
                "q": np.ascontiguousarray(q[b, :, hs, :].reshape(T, C)),
                "k": np.ascontiguousarray(k[b, :, hs, :].reshape(T, C)),
                "v": np.ascontiguousarray(v[b, :, hs, :].reshape(T, C)),
            }
        )
    return maps


def gather_outputs(results):
    out = np.empty((B, T, H, D), dtype=np.float32)
    for c in range(8):
        b, hp = divmod(c, 4)
        out[b, :, 2 * hp : 2 * hp + 2, :] = results[c]["o"].reshape(T, HPC, D)
    return out


def kernel(q, k, v):
    from concourse.bass_utils import run_bass_kernel_spmd

    q = np.asarray(q, dtype=np.float32)
    k = np.asarray(k, dtype=np.float32)
    v = np.asarray(v, dtype=np.float32)
    nc = get_nc()
    maps = shard_inputs(q, k, v)
    res = run_bass_kernel_spmd(nc, maps, list(range(8)))
    return gather_outputs(res.results)


# revision 15
# speedup vs baseline: 2.0179x; 1.0066x over previous
"""Trainium2 Bass kernel for nn_CausalLinearSelfAttention_30013231464545.

Math note: the reference cumsums the [B,T,H,D,M] kv tensor over axis=-2,
which is the *D* axis (faithful to the original torch code), so
  kv_sum[b,t,h,d,m] = csD(kf)[b,t,h,d] * v[b,t,h,m]
and the whole module collapses to
  out[b,t,h,m] = (s / denom) * v[b,t,h,m]
with
  denom[b,t,h] = sum_d qf * cumsum_T(kf)      (true causal running key sum)
  s[b,t,h]     = sum_d qf * cumsum_D(kf)      (per-timestep D-prefix sum)
  qf = elu(q)+1 = min(exp(q), 1) + relu(q),  kf likewise.
(Validated vs the jax reference at rel err 2e-7 in fp32.)

Sharding: B*H = 16 (b,h) slices; each core takes one (b, head-pair) slice
[T=2048, 2*64] so DMA rows are 512B contiguous. No cross-core comm.

Per-core dataflow (T on partitions, (h,d) on free; fp16 on-chip, fp32 I/O):
  - feature maps: ACT exp; DVE relu + fused min/add (16-bit fast modes)
  - cumsum over T: per-tile triangular matmuls + a [1,FD] column-sum matmul
    whose tile-prefix (computed by a tiny DVE chain) is broadcast back with
    one rank-1 matmul
  - cumsum over D: ONE segmented DVE scan (state = mask*state + kf, mask=0
    at each 64-wide segment start)
  - dots over D: two fp16 TT multiplies + one multi-axis tensor_reduce
  - scale & output: reciprocal + one broadcast TT multiply
"""

import numpy as np
import sys

sys.path.insert(0, "/opt/trn_rl_repo")

B, T, H, D = 2, 2048, 8, 64
P = 128          # partitions (t per tile)
HPC = 2          # heads per core
C = HPC * D      # per-core free width = 128
NT = T // P      # 16 t-tiles per core
GT = 4           # t-tiles per group
NG = NT // GT    # 4 groups
FD = GT * C      # 512 free elements per group supertile
NSEG = GT * HPC  # 8 (tile, head) segments per group

_CACHE = {}


def _build_nc():
    import concourse.bass as bass
    import concourse.bacc as bacc
    import concourse.mybir as mybir
    from concourse import tile

    dt = mybir.dt
    f32 = dt.float32
    f16 = dt.float16
    Alu = mybir.AluOpType
    Act = mybir.ActivationFunctionType

    nc = bacc.Bacc(None)

    q_d = nc.declare_dram_parameter("q", [T, C], f32, isOutput=False)
    k_d = nc.declare_dram_parameter("k", [T, C], f32, isOutput=False)
    v_d = nc.declare_dram_parameter("v", [T, C], f32, isOutput=False)
    o_d = nc.declare_dram_parameter("o", [T, C], f32, isOutput=True)

    # tri[t', t] = 1 if t' <= t  (lhsT for in-tile cumsum over partitions)
    tri_d = nc.inline_tensor(
        np.triu(np.ones((P, P), dtype=np.float16)), name="tri_const"
    )
    # segmented-scan reset mask: 0 at the first column of each 64-seg
    mask_np = np.ones((P, FD), dtype=np.float16)
    mask_np[:, 0::D] = 0.0
    mask_d = nc.inline_tensor(mask_np, name="mask_const")

    with tile.TileContext(nc) as tc:
        with (
            tc.tile_pool(name="const", bufs=1) as cpool,
            tc.tile_pool(name="io", bufs=3) as io,
            tc.tile_pool(name="wk", bufs=2) as wk,
            tc.tile_pool(name="cr", bufs=2) as crp,
            tc.tile_pool(name="ps", bufs=2, space="PSUM") as pp,
        ):
            tri_t = cpool.tile([P, P], f16, tag="tri")
            nc.sync.dma_start(tri_t[:], tri_d[:])
            mask_t = cpool.tile([P, FD], f16, tag="mask")
            nc.sync.dma_start(mask_t[:], mask_d[:])
            ones_t = cpool.tile([P, 1], f16, tag="ones")
            nc.vector.memset(ones_t[:], 1.0)
            onerow_t = cpool.tile([1, P], f16, tag="onerow")
            nc.vector.memset(onerow_t[:], 1.0)

            prev_rrow = None  # [1, FD+C]: blocks R0..R3 | next-group carry

            for g in range(NG):
                rows = slice(g * GT * P, (g + 1) * GT * P)
                qv = q_d[rows, :].rearrange("(j p) c -> p j c", p=P)
                kv = k_d[rows, :].rearrange("(j p) c -> p j c", p=P)
                vv = v_d[rows, :].rearrange("(j p) c -> p j c", p=P)

                qt = io.tile([P, FD], f32, tag="q")
                kt = io.tile([P, FD], f32, tag="k")
                vt = io.tile([P, FD], f32, tag="v")
                nc.sync.dma_start(qt[:].rearrange("p (j c) -> p j c", c=C), qv)
                nc.sync.dma_start(kt[:].rearrange("p (j c) -> p j c", c=C), kv)
                nc.sync.dma_start(vt[:].rearrange("p (j c) -> p j c", c=C), vv)

                # feature maps: f = min(exp(x), 1) + relu(x)  (== elu(x)+1)
                eq = wk.tile([P, FD], f16, tag="eq")
                ek = wk.tile([P, FD], f16, tag="ek")
                nc.scalar.activation(eq[:], qt[:], Act.Exp)
                nc.scalar.activation(ek[:], kt[:], Act.Exp)
                rq = wk.tile([P, FD], f16, tag="rq")
                rk = wk.tile([P, FD], f16, tag="rk")
                nc.vector.tensor_scalar_max(rq[:], qt[:], 0.0)
                nc.vector.tensor_scalar_max(rk[:], kt[:], 0.0)
                qf = wk.tile([P, FD], f16, tag="qf")
                kf = wk.tile([P, FD], f16, tag="kf")
                nc.vector.scalar_tensor_tensor(
                    qf[:], eq[:], 1.0, rq[:], op0=Alu.min, op1=Alu.add
                )
                nc.vector.scalar_tensor_tensor(
                    kf[:], ek[:], 1.0, rk[:], op0=Alu.min, op1=Alu.add
                )

                # per-tile column sums -> cs4 [1, FD] (PSUM)
                cs4 = pp.tile([1, FD], f32, tag="cs4")
                nc.tensor.matmul(cs4[:], ones_t[:], kf[:], start=True, stop=True)

                # rrow blocks: R_j = carry + sum_{jp<j} colsum_jp (fp16 chain);
                # block 4 = carry for the next group
                rrow = crp.tile([1, FD + C], f16, tag="rrow")
                if g == 0:
                    nc.vector.memset(rrow[:, 0:C], 0.0)
                else:
                    nc.vector.tensor_copy(rrow[:, 0:C], prev_rrow[:, FD : FD + C])
                for j in range(1, GT + 1):
                    nc.vector.tensor_tensor(
                        rrow[:, j * C : (j + 1) * C],
                        rrow[:, (j - 1) * C : j * C],
                        cs4[:, (j - 1) * C : j * C],
                        op=Alu.add,
                    )
                prev_rrow = rrow

                # ---- cumsum over T into PSUM (one accumulation group/bank) --
                ks = pp.tile([P, FD], f32, tag="ks")
                for j in range(GT):
                    sl = slice(j * C, (j + 1) * C)
                    nc.tensor.matmul(
                        ks[:, sl], tri_t[:], kf[:, sl],
                        start=(j == 0), stop=False,
                    )
                # add per-tile base rows via one rank-1 broadcast matmul
                nc.tensor.matmul(
                    ks[:], onerow_t[:], rrow[:, 0:FD], start=False, stop=True
                )
                ks_sb = wk.tile([P, FD], f16, tag="ks_sb")
                nc.scalar.copy(ks_sb[:], ks[:])

                # ---- cumsum over D: one segmented scan ----
                csDm = wk.tile([P, FD], f16, tag="csDm")
                nc.vector.tensor_tensor_scan(
                    csDm[:], mask_t[:], kf[:], 0.0, op0=Alu.mult, op1=Alu.add
                )

                # ---- dots over D: fp16 mults + one multi-axis reduce ----
                scr = wk.tile([P, 2 * FD], f16, tag="scr")
                nc.vector.tensor_tensor(scr[:, 0:FD], qf[:], ks_sb[:], op=Alu.mult)
                nc.vector.tensor_tensor(
                    scr[:, FD : 2 * FD], qf[:], csDm[:], op=Alu.mult
                )
                dn = wk.tile([P, 2 * NSEG], f32, tag="dn")
                nc.vector.tensor_reduce(
                    dn[:],
                    scr[:].rearrange("p (s d) -> p s d", d=D),
                    axis=mybir.AxisListType.X, op=Alu.add,
                )

                # scale[t, seg] = s / denom
                rec = wk.tile([P, NSEG], f32, tag="rec")
                nc.vector.reciprocal(rec[:], dn[:, 0:NSEG])
                sc = wk.tile([P, NSEG], f32, tag="sc")
                nc.vector.tensor_tensor(
                    sc[:], dn[:, NSEG : 2 * NSEG], rec[:], op=Alu.mult
                )

                # out = v * scale (broadcast over each 64-wide segment)
                ot = io.tile([P, FD], f32, tag="o")
                sc_b = sc[:].rearrange("p (s one) -> p s one", one=1).broadcast_to(
                    [P, NSEG, D]
                )
                nc.vector.tensor_tensor(
                    ot[:].rearrange("p (s d) -> p s d", d=D),
                    vt[:].rearrange("p (s d) -> p s d", d=D),
                    sc_b,
                    op=Alu.mult,
                )
                ov = o_d[rows, :].rearrange("(j p) c -> p j c", p=P)
                nc.sync.dma_start(ov, ot[:].rearrange("p (j c) -> p j c", c=C))

    nc.compile()
    return nc


def get_nc():
    if "nc" not in _CACHE:
        _CACHE["nc"] = _build_nc()
    return _CACHE["nc"]


def shard_inputs(q, k, v):
    """core c -> (b = c//4, heads 2*(c%4), 2*(c%4)+1); returns list of in_maps."""
    maps = []
    for c in range(8):
        b, hp = divmod(c, 4)
        hs = slice(2 * hp, 2 * hp + 2)
        maps.append(
            {
                "q": np.ascontiguousarray(q[b, :, hs, :].reshape(T, C)),
                "k": np.ascontiguousarray(k[b, :, hs, :].reshape(T, C)),
                "v": np.ascontiguousarray(v[b, :, hs, :].reshape(T, C)),
            }
        )
    return maps


def gather_outputs(results):
    out = np.empty((B, T, H, D), dtype=np.float32)
    for c in range(8):
        b, hp = divmod(c, 4)
        out[b, :, 2 * hp : 2 * hp + 2, :] = results[c]["o"].reshape(T, HPC, D)
    return out


def kernel(q, k, v):
    from concourse.bass_utils import run_bass_kernel_spmd

    q = np.asarray(q, dtype=np.float32)
    k = np.asarray(k, dtype=np.float32)
    v = np.asarray(v, dtype=np.float32)
    nc = get_nc()
    maps = shard_inputs(q, k, v)
    res = run_bass_kernel_spmd(nc, maps, list(range(8)))
    return gather_outputs(res.results)
